# revision 28
# baseline (speedup 1.0000x reference)
"""Multi-head attention (B=2, S=2048, D=1024, H=16) on 8 TRN2 NeuronCores.

Sharding: core c -> (batch b = c//4, head-group g = c%4 of 4 heads).
Each core computes, for its batch and 4 heads:
    Q/K/V projections, scores softmax (scaled by 1/sqrt(S)), attention
    output, and its partial slice of the output projection.
Host sums the 4 head-group partials per batch.

All device tensors are bf16 (PSUM accumulation stays fp32): halves DMA,
LDWEIGHTS and SBUF-stream traffic vs f32r at the same 1-cycle/row PE
rate. Structure:
  - Q^T/K^T [j, s] from lhsT=W chunks, rhs=X^T chunks (dc-outer waves
    chasing the X^T chunk DMAs; first X^T chunk is split 4-ways so it
    lands across DMA queues and the first wave starts ~4us in)
  - V   [s, j] from lhsT=X^T chunks, rhs=W_V chunks
  - scores S^T [k, q] from lhsT=K^T, rhs=Q^T (K=64 head-pairs packed at
    array rows 0/64 via tile_position -> full-rate)
  - exp via ACT with fused 1/sqrt(S) scale, written bf16; a ones-column
    in V~ makes the PV matmul (M=65) also produce softmax denominators
  - previous step's 32 dense PV matmuls ride at the start of each step
  - normalize O^T columns with 1/z via DRAM-bounce reshape + DVE
  - output projection per completed query block rides inside the
    attention steps (PSUM borrowed from the score pool) and y (bf16)
    streams out via DMA as it is produced -- no write-out tail
"""

import sys

if "/opt/trn_rl_repo" not in sys.path:
    sys.path.insert(0, "/opt/trn_rl_repo")

import numpy as np
import ml_dtypes

B = 2
S = 2048
D = 1024
H = 16
DK = 64
NCORES = 8
HG = 4  # heads per core
J = HG * DK  # 256, per-core projection width
QB = 512  # query block
NQB = S // QB  # 4
NKC = S // 128  # 16 key chunks
NDC = D // 128  # 8 contraction chunks
NJC = J // 128  # 2
SCALE_INV = float(1.0 / np.sqrt(np.float32(S)))

_CACHE = {}
LAST_RESULT = None


def _build():
    import concourse.mybir as mybir
    import concourse.tile as tile
    from concourse import bacc

    f32 = mybir.dt.float32
    bf16 = mybir.dt.bfloat16

    nc = bacc.Bacc("TRN2", target_bir_lowering=False, debug=False)

    xt_d = nc.declare_dram_parameter("xt", [D, S], bf16, isOutput=False)
    wq_d = nc.declare_dram_parameter("wq", [D, J], bf16, isOutput=False)
    wk_d = nc.declare_dram_parameter("wk", [D, J], bf16, isOutput=False)
    wv_d = nc.declare_dram_parameter("wv", [D, J], bf16, isOutput=False)
    w0_d = nc.declare_dram_parameter("w0", [J, D], bf16, isOutput=False)
    y_d = nc.declare_dram_parameter("y", [S, D], bf16, isOutput=True)

    with tile.TileContext(nc) as tc:
        with tc.tile_pool(name="persist", bufs=1) as A:
            # persistent tiles
            qt_t = A.tile([128, NJC, S], bf16)  # Q^T  [j, q]
            kt_t = A.tile([128, NJC, S], bf16)  # K^T  [j, k]
            v_t = A.tile([128, NKC, HG, DK + 1], bf16)  # V~ per head + ones
            ot_t = A.tile([128, NJC, S], bf16)  # O^T scaled  [j, q]
            w0_t = A.tile([128, NJC, D], bf16)
            ones_t = A.tile([128, NKC * HG], bf16)
            nc.vector.memset(ones_t, 1.0)
            nc.vector.tensor_copy(out=v_t[:, :, :, DK : DK + 1], in_=ones_t)

            # ---- phase 1: load X^T / W and project ----
            # dc-outer accumulation over 8 PSUM banks so the matmul waves
            # chase the X^T chunk DMAs instead of waiting for the full
            # transfer.
            with (
                tc.tile_pool(name="ph1", bufs=1) as Bp,
                tc.tile_pool(name="ps1", bufs=8, space="PSUM") as psA,
            ):
                xt_t = Bp.tile([128, NDC, S], bf16)
                wq_t = Bp.tile([128, NDC, J], bf16)
                wk_t = Bp.tile([128, NDC, J], bf16)
                wv_t = Bp.tile([128, NDC, J], bf16)
                wq_src = wq_d.ap().rearrange("(c p) j -> p c j", p=128)
                wk_src = wk_d.ap().rearrange("(c p) j -> p c j", p=128)
                xt_src = xt_d.ap().rearrange("(c p) q -> p c q", p=128)
                # First X^T chunk split 4-ways (spreads across DMA queues
                # so dc=0 arrives ~4x sooner); first wq chunk next; the
                # rest issued dc-major so arrival stays sequential.
                for q4 in range(4):
                    sl = slice(q4 * 512, (q4 + 1) * 512)
                    nc.sync.dma_start(out=xt_t[:, 0, sl], in_=xt_src[:, 0, sl])
                nc.sync.dma_start(out=wq_t[:, 0], in_=wq_src[:, 0])
                for dc in range(1, NDC):
                    nc.sync.dma_start(out=wq_t[:, dc], in_=wq_src[:, dc])
                    nc.sync.dma_start(out=xt_t[:, dc], in_=xt_src[:, dc])
                for dc in range(NDC):
                    nc.sync.dma_start(out=wk_t[:, dc], in_=wk_src[:, dc])
                nc.sync.dma_start(
                    out=wv_t, in_=wv_d.ap().rearrange("(c p) j -> p c j", p=128)
                )
                # w0 is not needed until the first output-projection block
                # (~halfway through) -- load it last.
                nc.sync.dma_start(
                    out=w0_t, in_=w0_d.ap().rearrange("(c p) m -> p c m", p=128)
                )

                for w_t, dst in ((wq_t, qt_t), (wk_t, kt_t)):
                    tiles = [
                        psA.tile([128, QB], f32, tag="p1", name=f"p1_{i}")
                        for i in range(8)
                    ]
                    for dc in range(NDC):
                        for idx in range(8):
                            jc, qb = idx // NQB, idx % NQB
                            nc.tensor.matmul(
                                tiles[idx],
                                w_t[:, dc, jc * 128 : (jc + 1) * 128],
                                xt_t[:, dc, qb * QB : (qb + 1) * QB],
                                start=(dc == 0),
                                stop=(dc == NDC - 1),
                            )
                    for idx in range(8):
                        jc, qb = idx // NQB, idx % NQB
                        o_ap = dst[:, jc, qb * QB : (qb + 1) * QB]
                        if idx % 2 == 0:
                            nc.vector.tensor_copy(out=o_ap, in_=tiles[idx])
                        else:
                            nc.scalar.activation(
                                out=o_ap,
                                in_=tiles[idx],
                                func=mybir.ActivationFunctionType.Copy,
                                scale=1.0,
                            )
                for wave in range(2):
                    tiles = [
                        psA.tile([128, QB], f32, tag="p1", name=f"p1_{i}")
                        for i in range(8)
                    ]
                    for dc in range(NDC):
                        for idx in range(8):
                            sc = wave * 8 + idx
                            nc.tensor.matmul(
                                tiles[idx][:, 0:J],
                                xt_t[:, dc, sc * 128 : (sc + 1) * 128],
                                wv_t[:, dc, :],
                                start=(dc == 0),
                                stop=(dc == NDC - 1),
                            )
                    for idx in range(8):
                        sc = wave * 8 + idx
                        o_ap = v_t[:, sc, :, 0:DK]
                        i_ap = tiles[idx][:, 0:J].rearrange(
                            "p (h d) -> p h d", h=HG
                        )
                        if idx % 2 == 0:
                            nc.vector.tensor_copy(out=o_ap, in_=i_ap)
                        else:
                            nc.scalar.activation(
                                out=o_ap,
                                in_=i_ap,
                                func=mybir.ActivationFunctionType.Copy,
                                scale=1.0,
                            )

            # ---- phase 2+3: attention with interleaved output proj ----
            # Steps are (qb, hp) head-PAIRS, 8 total. Per step, the 32
            # score chunks (16 kc x 2 heads, interleaved kcA,kcB,...) are
            # row-packed pairs (K=64 at base partitions 0/64 run
            # concurrently at full-array rate). Score PSUM tiles hold 3
            # chunks (6 banks double-buffered) so one ACT exp op covers
            # 1536 elements and the exp stream runs back-to-back. exp
            # results go to a per-step persistent expst tile; the previous
            # step's 32 PV matmuls (dense K=128 full-array work) are
            # emitted as a clump at the start of the next step, in chunk
            # order so expst chunks free up for the incoming exp stream.
            # After normalize of (qb, hp=1), the output projection for
            # that qb rides along (PSUM borrowed from the score pool) and
            # its y slice DMAs out immediately.
            with (
                tc.tile_pool(name="work", bufs=1) as C,
                tc.tile_pool(name="nrm", bufs=2) as Cn,
                tc.tile_pool(name="ytile", bufs=4) as Cy,
                tc.tile_pool(name="dbounce", bufs=2, space="DRAM") as Cd,
                tc.tile_pool(name="ps_s", bufs=2, space="PSUM") as psS,
                tc.tile_pool(name="ps_o", bufs=1, space="PSUM") as psO,
            ):
                NCH = 2 * NKC  # 32 score chunks per step
                GRPS = [(0, 2), (2, 5), (5, 8), (8, 11), (11, 14),
                        (14, 17), (17, 20), (20, 23), (23, 26), (26, 29),
                        (29, 32)]

                def emit_pv(qb, hp, expst):
                    """Dense PV clump: 32 K=128 matmuls in chunk order."""
                    ps_oa = psO.tile([128, QB], f32, tag="oa")
                    ps_ob = psO.tile([128, QB], f32, tag="ob")
                    for c in range(NCH):
                        kc, hb = c // 2, c % 2
                        ps_o = ps_oa if hb == 0 else ps_ob
                        nc.tensor.matmul(
                            ps_o[0 : DK + 1, :],
                            v_t[:, kc, 2 * hp + hb, :],
                            expst[:, c, :],
                            start=(kc == 0),
                            stop=(kc == NKC - 1),
                        )
                    return ps_oa, ps_ob

                def emit_normalize(qb, hp, ps_oa, ps_ob):
                    # Copy O rows to SBUF immediately (releases the PSUM
                    # accumulator); 1/z via the single-pass DVE
                    # reciprocal_approx_fast (18-bit, plenty for bf16
                    # outputs), broadcast across partitions on the idle
                    # GPSIMD. All engine ops -- the DRAM-bounce DMA chain
                    # used before raced DMA-write vs DMA-read on hw.
                    q_sl = slice(qb * QB, (qb + 1) * QB)
                    for p0, ps_o in ((0, ps_oa), (64, ps_ob)):
                        o_sb = Cn.tile([DK, QB], f32, tag=f"osb{p0}")
                        nc.vector.tensor_copy(o_sb, ps_o[0:DK, :])
                        z_sb = Cn.tile([1, QB], f32, tag=f"zs{p0}")
                        nc.vector.tensor_copy(z_sb, ps_o[DK : DK + 1, :])
                        r_sb = Cn.tile([1, QB], f32, tag=f"rs{p0}")
                        nc.vector.reciprocal_approx_fast(out=r_sb, in_=z_sb)
                        r_b = Cn.tile([64, QB], f32, tag=f"rb{p0}")
                        nc.gpsimd.partition_broadcast(r_b, r_sb)
                        nc.vector.tensor_mul(
                            ot_t[p0 : p0 + 64, hp, q_sl],
                            o_sb,
                            r_b,
                        )

                def emit_outproj(qb, lo=0, hi=8, copies_on_act=False):
                    """Output projection for (part of) a completed query
                    block: (qc, mb) slots [lo, hi) in groups of 3 using
                    score-pool PSUM; y slices stream to DRAM as they are
                    produced."""
                    slots = [
                        (qb * NQB + qc4, mb)
                        for qc4 in range(QB // 128)
                        for mb in range(D // QB)
                    ][lo:hi]
                    for g0 in range(0, len(slots), 3):
                        grp = slots[g0 : g0 + 3]
                        ps = psS.tile([128, 3, QB], f32, tag="s", name="ps_s")
                        for i, (qc, mb) in enumerate(grp):
                            for jc in range(NJC):
                                nc.tensor.matmul(
                                    ps[:, i],
                                    ot_t[:, jc, qc * 128 : (qc + 1) * 128],
                                    w0_t[:, jc, mb * QB : (mb + 1) * QB],
                                    start=(jc == 0),
                                    stop=(jc == NJC - 1),
                                )
                        for i, (qc, mb) in enumerate(grp):
                            y_t = Cy.tile([128, QB], bf16, tag="yt")
                            if copies_on_act and (qc + mb) % 2 == 0:
                                nc.scalar.activation(
                                    out=y_t,
                                    in_=ps[:, i],
                                    func=mybir.ActivationFunctionType.Copy,
                                    scale=1.0,
                                )
                            else:
                                nc.vector.tensor_copy(out=y_t, in_=ps[:, i])
                            # tail slots spread their write-out across three
                            # DGE queues so the final drain isn't serialized
                            # on one queue
                            dma_eng = (
                                (nc.sync, nc.scalar, nc.gpsimd)[(qc + mb) % 3]
                                if copies_on_act
                                else nc.sync
                            )
                            dma_eng.dma_start(
                                out=y_d.ap()[
                                    qc * 128 : (qc + 1) * 128,
                                    mb * QB : (mb + 1) * QB,
                                ],
                                in_=y_t,
                            )

                def emit_step(qb, hp, prev, final=False):
                    """Scores+exp stream for (qb, hp); the previous step's
                    PV clump rides at the start so the ACT exp stream
                    never pauses at step boundaries. After normalize of a
                    completed qb (hp=1), its output projection rides
                    along too. The final step additionally interleaves its
                    OWN PV chunks behind the exp stream (group g's PV runs
                    while group g+1's exp streams) so no PV clump is left
                    for a serial tail."""
                    q_sl = slice(qb * QB, (qb + 1) * QB)
                    expst = C.tile([128, NCH, QB], bf16, tag="expst")
                    prev_done = prev
                    if prev is not None:
                        pq, php, pexp = prev
                        ps_oa = psO.tile([128, QB], f32, tag="oa")
                        ps_ob = psO.tile([128, QB], f32, tag="ob")

                    def pv_all():
                        for c in range(NCH):
                            kc, hb = c // 2, c % 2
                            ps_o = ps_oa if hb == 0 else ps_ob
                            nc.tensor.matmul(
                                ps_o[0 : DK + 1, :],
                                v_t[:, kc, 2 * php + hb, :],
                                pexp[:, c, :],
                                start=(kc == 0),
                                stop=(kc == NKC - 1),
                            )

                    def score_group(g0, g1):
                        ps = psS.tile([128, 3, QB], f32, tag="s", name="ps_s")
                        for i, c in enumerate(range(g0, g1)):
                            kc, hb = c // 2, c % 2
                            p0 = hb * 64
                            k_sl = slice(kc * 128, (kc + 1) * 128)
                            nc.tensor.matmul(
                                ps[:, i],
                                kt_t[p0 : p0 + 64, hp, k_sl],
                                qt_t[p0 : p0 + 64, hp, q_sl],
                                start=True,
                                stop=True,
                                tile_position=(p0, 0),
                            )
                        nc.scalar.activation(
                            out=expst[:, g0:g1, :],
                            in_=ps[:, 0 : g1 - g0, :],
                            func=mybir.ActivationFunctionType.Exp,
                            scale=SCALE_INV,
                        )

                    def pv_prev(c0, c1):
                        for c in range(c0, min(c1, NCH)):
                            kc, hb = c // 2, c % 2
                            ps_o = ps_oa if hb == 0 else ps_ob
                            nc.tensor.matmul(
                                ps_o[0 : DK + 1, :],
                                v_t[:, kc, 2 * php + hb, :],
                                pexp[:, c, :],
                                start=(kc == 0),
                                stop=(kc == NKC - 1),
                            )

                    if not final:
                        # interleave the previous step's PV chunks between
                        # this step's score groups: the ACT exp stream gets
                        # fresh scores within ~1us of step start instead of
                        # idling behind a 32-matmul PV clump
                        done = 0
                        for gi, (g0, g1) in enumerate(GRPS):
                            score_group(g0, g1)
                            if prev is not None:
                                step_to = (gi + 1) * NCH // len(GRPS)
                                pv_prev(done, step_to)
                                done = step_to
                                if step_to >= NCH and gi < len(GRPS) - 1:
                                    prev_done = prev
                                    prev = None  # emitted; normalize below
                        if prev_done is not None:
                            emit_normalize(pq, php, ps_oa, ps_ob)
                        if prev_done is not None and php == 1:
                            # defer part of late query blocks' output
                            # projection into the final step, where it
                            # covers the last normalize chain's latency
                            hi = {0: 8, 1: 6, 2: 4}[pq]
                            emit_outproj(pq, 0, hi)
                        return expst

                    if prev is not None:
                        pv_all()
                        emit_normalize(pq, php, ps_oa, ps_ob)

                    # final step: chase the exp stream with our own PV
                    ps_fa = psO.tile([128, QB], f32, tag="oa")
                    ps_fb = psO.tile([128, QB], f32, tag="ob")

                    def pv_self(g0, g1):
                        for c in range(g0, g1):
                            kc, hb = c // 2, c % 2
                            ps_o = ps_fa if hb == 0 else ps_fb
                            nc.tensor.matmul(
                                ps_o[0 : DK + 1, :],
                                v_t[:, kc, 2 * hp + hb, :],
                                expst[:, c, :],
                                start=(kc == 0),
                                stop=(kc == NKC - 1),
                            )

                    for gi, (g0, g1) in enumerate(GRPS):
                        score_group(g0, g1)
                        if gi >= 1:
                            pv_self(*GRPS[gi - 1])
                    pv_self(*GRPS[-1])
                    emit_normalize(qb, hp, ps_fa, ps_fb)
                    # deferred late-block slots execute while the final z
                    # chain is in flight (y copies on the now-idle ACT)
                    emit_outproj(1, 6, 8, copies_on_act=True)
                    emit_outproj(2, 4, 8, copies_on_act=True)
                    emit_outproj(qb, copies_on_act=True)
                    return expst

                steps = [(qb, hp) for qb in range(NQB) for hp in range(2)]
                prev = None
                for i, (qb, hp) in enumerate(steps):
                    expst = emit_step(
                        qb, hp, prev, final=(i == len(steps) - 1)
                    )
                    prev = (qb, hp, expst)

    nc.compile()
    return nc


def kernel(X, W_Q, W_K, W_V, W_0):
    global LAST_RESULT
    from concourse.bass_utils import run_bass_kernel_spmd
    import os

    bf = ml_dtypes.bfloat16
    X = np.asarray(X, dtype=np.float32)
    W_Q = np.asarray(W_Q, dtype=np.float32).astype(bf)
    W_K = np.asarray(W_K, dtype=np.float32).astype(bf)
    W_V = np.asarray(W_V, dtype=np.float32).astype(bf)
    W_0 = np.asarray(W_0, dtype=np.float32).astype(bf)

    if "nc" not in _CACHE:
        _CACHE["nc"] = _build()
    nc = _CACHE["nc"]

    xt = [np.ascontiguousarray(X[b].T).astype(bf) for b in range(B)]
    in_maps = []
    for c in range(NCORES):
        b, g = c // HG, c % HG
        js = slice(g * J, (g + 1) * J)
        in_maps.append(
            {
                "xt": xt[b],
                "wq": np.ascontiguousarray(W_Q[:, js]),
                "wk": np.ascontiguousarray(W_K[:, js]),
                "wv": np.ascontiguousarray(W_V[:, js]),
                "w0": np.ascontiguousarray(W_0[js, :]),
            }
        )

    trace = bool(int(os.environ.get("KERNEL_TRACE", "0")))
    res = run_bass_kernel_spmd(
        nc, in_maps, list(range(NCORES)), trace=trace
    )
    LAST_RESULT = res

    out = np.zeros((B, S, D), dtype=np.float32)
    for c in range(NCORES):
        out[c // HG] += res.results[c]["y"].astype(np.float32)
    return out


# revision 29
# speedup vs baseline: 1.0073x; 1.0073x over previous
"""Multi-head attention (B=2, S=2048, D=1024, H=16) on 8 TRN2 NeuronCores.

Sharding: core c -> (batch b = c//4, head-group g = c%4 of 4 heads).
Each core computes, for its batch and 4 heads:
    Q/K/V projections, scores softmax (scaled by 1/sqrt(S)), attention
    output, and its partial slice of the output projection.
Host sums the 4 head-group partials per batch.

All device tensors are bf16 (PSUM accumulation stays fp32): halves DMA,
LDWEIGHTS and SBUF-stream traffic vs f32r at the same 1-cycle/row PE
rate. Structure:
  - Q^T/K^T [j, s] from lhsT=W chunks, rhs=X^T chunks (dc-outer waves
    chasing the X^T chunk DMAs; first X^T chunk is split 4-ways so it
    lands across DMA queues and the first wave starts ~4us in)
  - V   [s, j] from lhsT=X^T chunks, rhs=W_V chunks
  - scores S^T [k, q] from lhsT=K^T, rhs=Q^T (K=64 head-pairs packed at
    array rows 0/64 via tile_position -> full-rate)
  - exp via ACT with fused 1/sqrt(S) scale, written bf16; a ones-column
    in V~ makes the PV matmul (M=65) also produce softmax denominators
  - previous step's 32 dense PV matmuls ride at the start of each step
  - normalize O^T columns with 1/z via DRAM-bounce reshape + DVE
  - output projection per completed query block rides inside the
    attention steps (PSUM borrowed from the score pool) and y (bf16)
    streams out via DMA as it is produced -- no write-out tail
"""

import sys

if "/opt/trn_rl_repo" not in sys.path:
    sys.path.insert(0, "/opt/trn_rl_repo")

import numpy as np
import ml_dtypes

B = 2
S = 2048
D = 1024
H = 16
DK = 64
NCORES = 8
HG = 4  # heads per core
J = HG * DK  # 256, per-core projection width
QB = 512  # query block
NQB = S // QB  # 4
NKC = S // 128  # 16 key chunks
NDC = D // 128  # 8 contraction chunks
NJC = J // 128  # 2
SCALE_INV = float(1.0 / np.sqrt(np.float32(S)))

_CACHE = {}
LAST_RESULT = None


def _build():
    import concourse.mybir as mybir
    import concourse.tile as tile
    from concourse import bacc

    f32 = mybir.dt.float32
    bf16 = mybir.dt.bfloat16

    nc = bacc.Bacc("TRN2", target_bir_lowering=False, debug=False)

    xt_d = nc.declare_dram_parameter("xt", [D, S], bf16, isOutput=False)
    wq_d = nc.declare_dram_parameter("wq", [D, J], bf16, isOutput=False)
    wk_d = nc.declare_dram_parameter("wk", [D, J], bf16, isOutput=False)
    wv_d = nc.declare_dram_parameter("wv", [D, J], bf16, isOutput=False)
    w0_d = nc.declare_dram_parameter("w0", [J, D], bf16, isOutput=False)
    y_d = nc.declare_dram_parameter("y", [S, D], bf16, isOutput=True)

    with tile.TileContext(nc) as tc:
        with tc.tile_pool(name="persist", bufs=1) as A:
            # persistent tiles
            qt_t = A.tile([128, NJC, S], bf16)  # Q^T  [j, q]
            kt_t = A.tile([128, NJC, S], bf16)  # K^T  [j, k]
            v_t = A.tile([128, NKC, HG, DK + 1], bf16)  # V~ per head + ones
            ot_t = A.tile([128, NJC, S], bf16)  # O^T scaled  [j, q]
            w0_t = A.tile([128, NJC, D], bf16)
            ones_t = A.tile([128, NKC * HG], bf16)
            nc.vector.memset(ones_t, 1.0)
            nc.vector.tensor_copy(out=v_t[:, :, :, DK : DK + 1], in_=ones_t)

            # ---- phase 1: load X^T / W and project ----
            # dc-outer accumulation over 8 PSUM banks so the matmul waves
            # chase the X^T chunk DMAs instead of waiting for the full
            # transfer.
            with (
                tc.tile_pool(name="ph1", bufs=1) as Bp,
                tc.tile_pool(name="ps1", bufs=8, space="PSUM") as psA,
            ):
                xt_t = Bp.tile([128, NDC, S], bf16)
                wq_t = Bp.tile([128, NDC, J], bf16)
                wk_t = Bp.tile([128, NDC, J], bf16)
                wv_t = Bp.tile([128, NDC, J], bf16)
                wq_src = wq_d.ap().rearrange("(c p) j -> p c j", p=128)
                wk_src = wk_d.ap().rearrange("(c p) j -> p c j", p=128)
                xt_src = xt_d.ap().rearrange("(c p) q -> p c q", p=128)
                # First X^T chunk split 4-ways (spreads across DMA queues
                # so dc=0 arrives ~4x sooner); first wq chunk next; the
                # rest issued dc-major so arrival stays sequential.
                for q4 in range(4):
                    sl = slice(q4 * 512, (q4 + 1) * 512)
                    nc.sync.dma_start(out=xt_t[:, 0, sl], in_=xt_src[:, 0, sl])
                nc.sync.dma_start(out=wq_t[:, 0], in_=wq_src[:, 0])
                for dc in range(1, NDC):
                    nc.sync.dma_start(out=wq_t[:, dc], in_=wq_src[:, dc])
                    nc.sync.dma_start(out=xt_t[:, dc], in_=xt_src[:, dc])
                for dc in range(NDC):
                    nc.sync.dma_start(out=wk_t[:, dc], in_=wk_src[:, dc])
                nc.sync.dma_start(
                    out=wv_t, in_=wv_d.ap().rearrange("(c p) j -> p c j", p=128)
                )
                # w0 is not needed until the first output-projection block
                # (~halfway through) -- load it last.
                nc.sync.dma_start(
                    out=w0_t, in_=w0_d.ap().rearrange("(c p) m -> p c m", p=128)
                )

                for w_t, dst in ((wq_t, qt_t), (wk_t, kt_t)):
                    tiles = [
                        psA.tile([128, QB], f32, tag="p1", name=f"p1_{i}")
                        for i in range(8)
                    ]
                    for dc in range(NDC):
                        for idx in range(8):
                            jc, qb = idx // NQB, idx % NQB
                            nc.tensor.matmul(
                                tiles[idx],
                                w_t[:, dc, jc * 128 : (jc + 1) * 128],
                                xt_t[:, dc, qb * QB : (qb + 1) * QB],
                                start=(dc == 0),
                                stop=(dc == NDC - 1),
                            )
                    for idx in range(8):
                        jc, qb = idx // NQB, idx % NQB
                        o_ap = dst[:, jc, qb * QB : (qb + 1) * QB]
                        if idx % 2 == 0:
                            nc.vector.tensor_copy(out=o_ap, in_=tiles[idx])
                        else:
                            nc.scalar.activation(
                                out=o_ap,
                                in_=tiles[idx],
                                func=mybir.ActivationFunctionType.Copy,
                                scale=1.0,
                            )
                for wave in range(2):
                    tiles = [
                        psA.tile([128, QB], f32, tag="p1", name=f"p1_{i}")
                        for i in range(8)
                    ]
                    for dc in range(NDC):
                        for idx in range(8):
                            sc = wave * 8 + idx
                            nc.tensor.matmul(
                                tiles[idx][:, 0:J],
                                xt_t[:, dc, sc * 128 : (sc + 1) * 128],
                                wv_t[:, dc, :],
                                start=(dc == 0),
                                stop=(dc == NDC - 1),
                            )
                    for idx in range(8):
                        sc = wave * 8 + idx
                        o_ap = v_t[:, sc, :, 0:DK]
                        i_ap = tiles[idx][:, 0:J].rearrange(
                            "p (h d) -> p h d", h=HG
                        )
                        if idx % 2 == 0:
                            nc.vector.tensor_copy(out=o_ap, in_=i_ap)
                        else:
                            nc.scalar.activation(
                                out=o_ap,
                                in_=i_ap,
                                func=mybir.ActivationFunctionType.Copy,
                                scale=1.0,
                            )

            # ---- phase 2+3: attention with interleaved output proj ----
            # Steps are (qb, hp) head-PAIRS, 8 total. Per step, the 32
            # score chunks (16 kc x 2 heads, interleaved kcA,kcB,...) are
            # row-packed pairs (K=64 at base partitions 0/64 run
            # concurrently at full-array rate). Score PSUM tiles hold 3
            # chunks (6 banks double-buffered) so one ACT exp op covers
            # 1536 elements and the exp stream runs back-to-back. exp
            # results go to a per-step persistent expst tile; the previous
            # step's 32 PV matmuls (dense K=128 full-array work) are
            # emitted as a clump at the start of the next step, in chunk
            # order so expst chunks free up for the incoming exp stream.
            # After normalize of (qb, hp=1), the output projection for
            # that qb rides along (PSUM borrowed from the score pool) and
            # its y slice DMAs out immediately.
            with (
                tc.tile_pool(name="work", bufs=1) as C,
                tc.tile_pool(name="nrm", bufs=2) as Cn,
                tc.tile_pool(name="ytile", bufs=4) as Cy,
                tc.tile_pool(name="dbounce", bufs=2, space="DRAM") as Cd,
                tc.tile_pool(name="ps_s", bufs=2, space="PSUM") as psS,
                tc.tile_pool(name="ps_o", bufs=1, space="PSUM") as psO,
            ):
                NCH = 2 * NKC  # 32 score chunks per step
                GRPS = [(0, 2), (2, 5), (5, 8), (8, 11), (11, 14),
                        (14, 17), (17, 20), (20, 23), (23, 26), (26, 29),
                        (29, 32)]

                def emit_pv(qb, hp, expst):
                    """Dense PV clump: 32 K=128 matmuls in chunk order."""
                    ps_oa = psO.tile([128, QB], f32, tag="oa")
                    ps_ob = psO.tile([128, QB], f32, tag="ob")
                    for c in range(NCH):
                        kc, hb = c // 2, c % 2
                        ps_o = ps_oa if hb == 0 else ps_ob
                        nc.tensor.matmul(
                            ps_o[0 : DK + 1, :],
                            v_t[:, kc, 2 * hp + hb, :],
                            expst[:, c, :],
                            start=(kc == 0),
                            stop=(kc == NKC - 1),
                        )
                    return ps_oa, ps_ob

                def emit_normalize(qb, hp, ps_oa, ps_ob):
                    # Copy O rows to SBUF immediately (releases the PSUM
                    # accumulator); 1/z via the single-pass DVE
                    # reciprocal_approx_fast (18-bit, plenty for bf16
                    # outputs), broadcast across partitions on the idle
                    # GPSIMD. All engine ops -- the DRAM-bounce DMA chain
                    # used before raced DMA-write vs DMA-read on hw.
                    q_sl = slice(qb * QB, (qb + 1) * QB)
                    for p0, ps_o in ((0, ps_oa), (64, ps_ob)):
                        o_sb = Cn.tile([DK, QB], f32, tag=f"osb{p0}")
                        nc.vector.tensor_copy(o_sb, ps_o[0:DK, :])
                        z_sb = Cn.tile([1, QB], f32, tag=f"zs{p0}")
                        nc.vector.tensor_copy(z_sb, ps_o[DK : DK + 1, :])
                        r_sb = Cn.tile([1, QB], f32, tag=f"rs{p0}")
                        nc.vector.reciprocal_approx_fast(out=r_sb, in_=z_sb)
                        r_b = Cn.tile([64, QB], f32, tag=f"rb{p0}")
                        nc.gpsimd.partition_broadcast(r_b, r_sb)
                        nc.vector.tensor_mul(
                            ot_t[p0 : p0 + 64, hp, q_sl],
                            o_sb,
                            r_b,
                        )

                def emit_outproj(qb, lo=0, hi=8, copies_on_act=False):
                    """Output projection for (part of) a completed query
                    block: (qc, mb) slots [lo, hi) in groups of 3 using
                    score-pool PSUM; y slices stream to DRAM as they are
                    produced."""
                    slots = [
                        (qb * NQB + qc4, mb)
                        for qc4 in range(QB // 128)
                        for mb in range(D // QB)
                    ][lo:hi]
                    for g0 in range(0, len(slots), 3):
                        grp = slots[g0 : g0 + 3]
                        ps = psS.tile([128, 3, QB], f32, tag="s", name="ps_s")
                        for i, (qc, mb) in enumerate(grp):
                            for jc in range(NJC):
                                nc.tensor.matmul(
                                    ps[:, i],
                                    ot_t[:, jc, qc * 128 : (qc + 1) * 128],
                                    w0_t[:, jc, mb * QB : (mb + 1) * QB],
                                    start=(jc == 0),
                                    stop=(jc == NJC - 1),
                                )
                        for i, (qc, mb) in enumerate(grp):
                            y_t = Cy.tile([128, QB], bf16, tag="yt")
                            if copies_on_act and (qc + mb) % 2 == 0:
                                nc.scalar.activation(
                                    out=y_t,
                                    in_=ps[:, i],
                                    func=mybir.ActivationFunctionType.Copy,
                                    scale=1.0,
                                )
                            else:
                                nc.vector.tensor_copy(out=y_t, in_=ps[:, i])
                            # tail slots spread their write-out across three
                            # DGE queues so the final drain isn't serialized
                            # on one queue
                            dma_eng = (
                                (nc.sync, nc.scalar, nc.gpsimd)[(qc + mb) % 3]
                                if copies_on_act
                                else nc.sync
                            )
                            dma_eng.dma_start(
                                out=y_d.ap()[
                                    qc * 128 : (qc + 1) * 128,
                                    mb * QB : (mb + 1) * QB,
                                ],
                                in_=y_t,
                            )

                def emit_step(qb, hp, prev, final=False):
                    """Scores+exp stream for (qb, hp); the previous step's
                    PV clump rides at the start so the ACT exp stream
                    never pauses at step boundaries. After normalize of a
                    completed qb (hp=1), its output projection rides
                    along too. The final step additionally interleaves its
                    OWN PV chunks behind the exp stream (group g's PV runs
                    while group g+1's exp streams) so no PV clump is left
                    for a serial tail."""
                    q_sl = slice(qb * QB, (qb + 1) * QB)
                    expst = C.tile([128, NCH, QB], bf16, tag="expst")
                    if prev is not None:
                        pq, php, pexp = prev
                        ps_oa = psO.tile([128, QB], f32, tag="oa")
                        ps_ob = psO.tile([128, QB], f32, tag="ob")

                    def pv_all():
                        for c in range(NCH):
                            kc, hb = c // 2, c % 2
                            ps_o = ps_oa if hb == 0 else ps_ob
                            nc.tensor.matmul(
                                ps_o[0 : DK + 1, :],
                                v_t[:, kc, 2 * php + hb, :],
                                pexp[:, c, :],
                                start=(kc == 0),
                                stop=(kc == NKC - 1),
                            )

                    def score_group(g0, g1):
                        ps = psS.tile([128, 3, QB], f32, tag="s", name="ps_s")
                        for i, c in enumerate(range(g0, g1)):
                            kc, hb = c // 2, c % 2
                            p0 = hb * 64
                            k_sl = slice(kc * 128, (kc + 1) * 128)
                            nc.tensor.matmul(
                                ps[:, i],
                                kt_t[p0 : p0 + 64, hp, k_sl],
                                qt_t[p0 : p0 + 64, hp, q_sl],
                                start=True,
                                stop=True,
                                tile_position=(p0, 0),
                            )
                        nc.scalar.activation(
                            out=expst[:, g0:g1, :],
                            in_=ps[:, 0 : g1 - g0, :],
                            func=mybir.ActivationFunctionType.Exp,
                            scale=SCALE_INV,
                        )

                    if prev is not None:
                        pv_all()
                        # normalize immediately: its short engine-only chain
                        # runs under the score stream, so the out-proj
                        # matmuls emitted later never wait on it
                        emit_normalize(pq, php, ps_oa, ps_ob)

                    if not final:
                        for g0, g1 in GRPS:
                            score_group(g0, g1)
                        if prev is not None and php == 1:
                            # defer part of late query blocks' output
                            # projection into the final step, where it
                            # covers the last normalize chain's latency
                            hi = {0: 8, 1: 6, 2: 4}[pq]
                            emit_outproj(pq, 0, hi)
                        return expst

                    # final step: chase the exp stream with our own PV
                    ps_fa = psO.tile([128, QB], f32, tag="oa")
                    ps_fb = psO.tile([128, QB], f32, tag="ob")

                    def pv_self(g0, g1):
                        for c in range(g0, g1):
                            kc, hb = c // 2, c % 2
                            ps_o = ps_fa if hb == 0 else ps_fb
                            nc.tensor.matmul(
                                ps_o[0 : DK + 1, :],
                                v_t[:, kc, 2 * hp + hb, :],
                                expst[:, c, :],
                                start=(kc == 0),
                                stop=(kc == NKC - 1),
                            )

                    for gi, (g0, g1) in enumerate(GRPS):
                        score_group(g0, g1)
                        if gi >= 1:
                            pv_self(*GRPS[gi - 1])
                    pv_self(*GRPS[-1])
                    emit_normalize(qb, hp, ps_fa, ps_fb)
                    # deferred late-block slots execute while the final z
                    # chain is in flight (y copies on the now-idle ACT)
                    emit_outproj(1, 6, 8, copies_on_act=True)
                    emit_outproj(2, 4, 8, copies_on_act=True)
                    emit_outproj(qb, copies_on_act=True)
                    return expst

                steps = [(qb, hp) for qb in range(NQB) for hp in range(2)]
                prev = None
                for i, (qb, hp) in enumerate(steps):
                    expst = emit_step(
                        qb, hp, prev, final=(i == len(steps) - 1)
                    )
                    prev = (qb, hp, expst)

    nc.compile()
    return nc


def kernel(X, W_Q, W_K, W_V, W_0):
    global LAST_RESULT
    from concourse.bass_utils import run_bass_kernel_spmd
    import os

    bf = ml_dtypes.bfloat16
    X = np.asarray(X, dtype=np.float32)
    W_Q = np.asarray(W_Q, dtype=np.float32).astype(bf)
    W_K = np.asarray(W_K, dtype=np.float32).astype(bf)
    W_V = np.asarray(W_V, dtype=np.float32).astype(bf)
    W_0 = np.asarray(W_0, dtype=np.float32).astype(bf)

    if "nc" not in _CACHE:
        _CACHE["nc"] = _build()
    nc = _CACHE["nc"]

    xt = [np.ascontiguousarray(X[b].T).astype(bf) for b in range(B)]
    in_maps = []
    for c in range(NCORES):
        b, g = c // HG, c % HG
        js = slice(g * J, (g + 1) * J)
        in_maps.append(
            {
                "xt": xt[b],
                "wq": np.ascontiguousarray(W_Q[:, js]),
                "wk": np.ascontiguousarray(W_K[:, js]),
                "wv": np.ascontiguousarray(W_V[:, js]),
                "w0": np.ascontiguousarray(W_0[js, :]),
            }
        )

    trace = bool(int(os.environ.get("KERNEL_TRACE", "0")))
    res = run_bass_kernel_spmd(
        nc, in_maps, list(range(NCORES)), trace=trace
    )
    LAST_RESULT = res

    out = np.zeros((B, S, D), dtype=np.float32)
    for c in range(NCORES):
        out[c // HG] += res.results[c]["y"].astype(np.float32)
    return out


# revision 30
# speedup vs baseline: 1.0272x; 1.0198x over previous
"""Multi-head attention (B=2, S=2048, D=1024, H=16) on 8 TRN2 NeuronCores.

Sharding: core c -> (batch b = c//4, head-group g = c%4 of 4 heads).
Each core computes, for its batch and 4 heads:
    Q/K/V projections, scores softmax (scaled by 1/sqrt(S)), attention
    output, and its partial slice of the output projection.
Host sums the 4 head-group partials per batch.

All device tensors are bf16 (PSUM accumulation stays fp32): halves DMA,
LDWEIGHTS and SBUF-stream traffic vs f32r at the same 1-cycle/row PE
rate. Structure:
  - Q^T/K^T [j, s] from lhsT=W chunks, rhs=X^T chunks (dc-outer waves
    chasing the X^T chunk DMAs; first X^T chunk is split 4-ways so it
    lands across DMA queues and the first wave starts ~4us in)
  - V   [s, j] from lhsT=X^T chunks, rhs=W_V chunks
  - scores S^T [k, q] from lhsT=K^T, rhs=Q^T (K=64 head-pairs packed at
    array rows 0/64 via tile_position -> full-rate)
  - exp via ACT with fused 1/sqrt(S) scale, written bf16; a ones-column
    in V~ makes the PV matmul (M=65) also produce softmax denominators
  - previous step's 32 dense PV matmuls ride at the start of each step
  - normalize O^T columns with 1/z via DRAM-bounce reshape + DVE
  - output projection per completed query block rides inside the
    attention steps (PSUM borrowed from the score pool) and y (bf16)
    streams out via DMA as it is produced -- no write-out tail
"""

import sys

if "/opt/trn_rl_repo" not in sys.path:
    sys.path.insert(0, "/opt/trn_rl_repo")

import numpy as np
import ml_dtypes

B = 2
S = 2048
D = 1024
H = 16
DK = 64
NCORES = 8
HG = 4  # heads per core
J = HG * DK  # 256, per-core projection width
QB = 512  # query block
NQB = S // QB  # 4
NKC = S // 128  # 16 key chunks
NDC = D // 128  # 8 contraction chunks
NJC = J // 128  # 2
SCALE_INV = float(1.0 / np.sqrt(np.float32(S)))

_CACHE = {}
LAST_RESULT = None


def _build():
    import concourse.mybir as mybir
    import concourse.tile as tile
    from concourse import bacc

    f32 = mybir.dt.float32
    bf16 = mybir.dt.bfloat16

    nc = bacc.Bacc("TRN2", target_bir_lowering=False, debug=False)

    xt_d = nc.declare_dram_parameter("xt", [D, S], bf16, isOutput=False)
    wq_d = nc.declare_dram_parameter("wq", [D, J], bf16, isOutput=False)
    wk_d = nc.declare_dram_parameter("wk", [D, J], bf16, isOutput=False)
    wv_d = nc.declare_dram_parameter("wv", [D, J], bf16, isOutput=False)
    w0_d = nc.declare_dram_parameter("w0", [J, D], bf16, isOutput=False)
    y_d = nc.declare_dram_parameter("y", [S, D], bf16, isOutput=True)

    with tile.TileContext(nc) as tc:
        with tc.tile_pool(name="persist", bufs=1) as A:
            # persistent tiles
            qt_t = A.tile([128, NJC, S], bf16)  # Q^T  [j, q]
            kt_t = A.tile([128, NJC, S], bf16)  # K^T  [j, k]
            v_t = A.tile([128, NKC, HG, DK + 1], bf16)  # V~ per head + ones
            ot_t = A.tile([128, NJC, S], bf16)  # O^T scaled  [j, q]
            w0_t = A.tile([128, NJC, D], bf16)
            ones_t = A.tile([128, NKC * HG], bf16)
            nc.vector.memset(ones_t, 1.0)
            nc.vector.tensor_copy(out=v_t[:, :, :, DK : DK + 1], in_=ones_t)

            # ---- phase 1: load X^T / W and project ----
            # dc-outer accumulation over 8 PSUM banks so the matmul waves
            # chase the X^T chunk DMAs instead of waiting for the full
            # transfer.
            with (
                tc.tile_pool(name="ph1", bufs=1) as Bp,
                tc.tile_pool(name="ps1", bufs=8, space="PSUM") as psA,
            ):
                xt_t = Bp.tile([128, NDC, S], bf16)
                wq_t = Bp.tile([128, NDC, J], bf16)
                wk_t = Bp.tile([128, NDC, J], bf16)
                wv_t = Bp.tile([128, NDC, J], bf16)
                wq_src = wq_d.ap().rearrange("(c p) j -> p c j", p=128)
                wk_src = wk_d.ap().rearrange("(c p) j -> p c j", p=128)
                xt_src = xt_d.ap().rearrange("(c p) q -> p c q", p=128)
                # First X^T chunk split 4-ways (spreads across DMA queues
                # so dc=0 arrives ~4x sooner); first wq chunk next; the
                # rest issued dc-major so arrival stays sequential.
                for q4 in range(4):
                    sl = slice(q4 * 512, (q4 + 1) * 512)
                    nc.sync.dma_start(out=xt_t[:, 0, sl], in_=xt_src[:, 0, sl])
                nc.sync.dma_start(out=wq_t[:, 0], in_=wq_src[:, 0])
                for dc in range(1, NDC):
                    nc.sync.dma_start(out=wq_t[:, dc], in_=wq_src[:, dc])
                    nc.sync.dma_start(out=xt_t[:, dc], in_=xt_src[:, dc])
                for dc in range(NDC):
                    nc.sync.dma_start(out=wk_t[:, dc], in_=wk_src[:, dc])
                nc.sync.dma_start(
                    out=wv_t, in_=wv_d.ap().rearrange("(c p) j -> p c j", p=128)
                )
                # w0 is not needed until the first output-projection block
                # (~halfway through) -- load it last.
                nc.sync.dma_start(
                    out=w0_t, in_=w0_d.ap().rearrange("(c p) m -> p c m", p=128)
                )

                for w_t, dst in ((wq_t, qt_t), (wk_t, kt_t)):
                    tiles = [
                        psA.tile([128, QB], f32, tag="p1", name=f"p1_{i}")
                        for i in range(8)
                    ]
                    for dc in range(NDC):
                        for idx in range(8):
                            jc, qb = idx // NQB, idx % NQB
                            nc.tensor.matmul(
                                tiles[idx],
                                w_t[:, dc, jc * 128 : (jc + 1) * 128],
                                xt_t[:, dc, qb * QB : (qb + 1) * QB],
                                start=(dc == 0),
                                stop=(dc == NDC - 1),
                            )
                    for idx in range(8):
                        jc, qb = idx // NQB, idx % NQB
                        o_ap = dst[:, jc, qb * QB : (qb + 1) * QB]
                        if idx % 2 == 0:
                            nc.vector.tensor_copy(out=o_ap, in_=tiles[idx])
                        else:
                            nc.scalar.activation(
                                out=o_ap,
                                in_=tiles[idx],
                                func=mybir.ActivationFunctionType.Copy,
                                scale=1.0,
                            )
                for wave in range(2):
                    tiles = [
                        psA.tile([128, QB], f32, tag="p1", name=f"p1_{i}")
                        for i in range(8)
                    ]
                    for dc in range(NDC):
                        for idx in range(8):
                            sc = wave * 8 + idx
                            nc.tensor.matmul(
                                tiles[idx][:, 0:J],
                                xt_t[:, dc, sc * 128 : (sc + 1) * 128],
                                wv_t[:, dc, :],
                                start=(dc == 0),
                                stop=(dc == NDC - 1),
                            )
                    for idx in range(8):
                        sc = wave * 8 + idx
                        o_ap = v_t[:, sc, :, 0:DK]
                        i_ap = tiles[idx][:, 0:J].rearrange(
                            "p (h d) -> p h d", h=HG
                        )
                        if idx % 2 == 0:
                            nc.vector.tensor_copy(out=o_ap, in_=i_ap)
                        else:
                            nc.scalar.activation(
                                out=o_ap,
                                in_=i_ap,
                                func=mybir.ActivationFunctionType.Copy,
                                scale=1.0,
                            )

            # ---- phase 2+3: attention with interleaved output proj ----
            # Steps are (qb, hp) head-PAIRS, 8 total. Per step, the 32
            # score chunks (16 kc x 2 heads, interleaved kcA,kcB,...) are
            # row-packed pairs (K=64 at base partitions 0/64 run
            # concurrently at full-array rate). Score PSUM tiles hold 3
            # chunks (6 banks double-buffered) so one ACT exp op covers
            # 1536 elements and the exp stream runs back-to-back. exp
            # results go to a per-step persistent expst tile; the previous
            # step's 32 PV matmuls (dense K=128 full-array work) are
            # emitted as a clump at the start of the next step, in chunk
            # order so expst chunks free up for the incoming exp stream.
            # After normalize of (qb, hp=1), the output projection for
            # that qb rides along (PSUM borrowed from the score pool) and
            # its y slice DMAs out immediately.
            with (
                tc.tile_pool(name="work", bufs=1) as C,
                tc.tile_pool(name="nrm", bufs=2) as Cn,
                tc.tile_pool(name="ytile", bufs=4) as Cy,
                tc.tile_pool(name="dbounce", bufs=2, space="DRAM") as Cd,
                tc.tile_pool(name="ps_s", bufs=2, space="PSUM") as psS,
                tc.tile_pool(name="ps_o", bufs=1, space="PSUM") as psO,
            ):
                NCH = 2 * NKC  # 32 score chunks per step
                GRPS = [(0, 2), (2, 5), (5, 8), (8, 11), (11, 14),
                        (14, 17), (17, 20), (20, 23), (23, 26), (26, 29),
                        (29, 32)]

                def emit_pv(qb, hp, expst):
                    """Dense PV clump: 32 K=128 matmuls in chunk order."""
                    ps_oa = psO.tile([128, QB], f32, tag="oa")
                    ps_ob = psO.tile([128, QB], f32, tag="ob")
                    for c in range(NCH):
                        kc, hb = c // 2, c % 2
                        ps_o = ps_oa if hb == 0 else ps_ob
                        nc.tensor.matmul(
                            ps_o[0 : DK + 1, :],
                            v_t[:, kc, 2 * hp + hb, :],
                            expst[:, c, :],
                            start=(kc == 0),
                            stop=(kc == NKC - 1),
                        )
                    return ps_oa, ps_ob

                def emit_normalize(qb, hp, ps_oa, ps_ob):
                    # Copy O rows to SBUF immediately (releases the PSUM
                    # accumulator); 1/z via the single-pass DVE
                    # reciprocal_approx_fast (18-bit, plenty for bf16
                    # outputs), broadcast across partitions on the idle
                    # GPSIMD. All engine ops -- the DRAM-bounce DMA chain
                    # used before raced DMA-write vs DMA-read on hw.
                    q_sl = slice(qb * QB, (qb + 1) * QB)
                    for p0, ps_o in ((0, ps_oa), (64, ps_ob)):
                        o_sb = Cn.tile([DK, QB], f32, tag=f"osb{p0}")
                        nc.vector.tensor_copy(o_sb, ps_o[0:DK, :])
                        z_sb = Cn.tile([1, QB], f32, tag=f"zs{p0}")
                        nc.vector.tensor_copy(z_sb, ps_o[DK : DK + 1, :])
                        r_sb = Cn.tile([1, QB], f32, tag=f"rs{p0}")
                        nc.vector.reciprocal_approx_fast(out=r_sb, in_=z_sb)
                        r_b = Cn.tile([64, QB], f32, tag=f"rb{p0}")
                        nc.gpsimd.partition_broadcast(r_b, r_sb)
                        nc.vector.tensor_mul(
                            ot_t[p0 : p0 + 64, hp, q_sl],
                            o_sb,
                            r_b,
                        )

                def emit_outproj(qb, lo=0, hi=8, copies_on_act=False):
                    """Output projection for (part of) a completed query
                    block: (qc, mb) slots [lo, hi) in groups of 3 using
                    score-pool PSUM; y slices stream to DRAM as they are
                    produced."""
                    slots = [
                        (qb * NQB + qc4, mb)
                        for qc4 in range(QB // 128)
                        for mb in range(D // QB)
                    ][lo:hi]
                    for g0 in range(0, len(slots), 3):
                        grp = slots[g0 : g0 + 3]
                        ps = psS.tile([128, 3, QB], f32, tag="s", name="ps_s")
                        for i, (qc, mb) in enumerate(grp):
                            for jc in range(NJC):
                                nc.tensor.matmul(
                                    ps[:, i],
                                    ot_t[:, jc, qc * 128 : (qc + 1) * 128],
                                    w0_t[:, jc, mb * QB : (mb + 1) * QB],
                                    start=(jc == 0),
                                    stop=(jc == NJC - 1),
                                )
                        for i, (qc, mb) in enumerate(grp):
                            y_t = Cy.tile([128, QB], bf16, tag="yt")
                            if copies_on_act:
                                nc.scalar.activation(
                                    out=y_t,
                                    in_=ps[:, i],
                                    func=mybir.ActivationFunctionType.Copy,
                                    scale=1.0,
                                )
                            else:
                                nc.vector.tensor_copy(out=y_t, in_=ps[:, i])
                            nc.sync.dma_start(
                                out=y_d.ap()[
                                    qc * 128 : (qc + 1) * 128,
                                    mb * QB : (mb + 1) * QB,
                                ],
                                in_=y_t,
                            )

                def emit_step(qb, hp, prev, final=False):
                    """Scores+exp stream for (qb, hp); the previous step's
                    PV clump rides at the start so the ACT exp stream
                    never pauses at step boundaries. After normalize of a
                    completed qb (hp=1), its output projection rides
                    along too. The final step additionally interleaves its
                    OWN PV chunks behind the exp stream (group g's PV runs
                    while group g+1's exp streams) so no PV clump is left
                    for a serial tail."""
                    q_sl = slice(qb * QB, (qb + 1) * QB)
                    expst = C.tile([128, NCH, QB], bf16, tag="expst")
                    if prev is not None:
                        pq, php, pexp = prev
                        ps_oa = psO.tile([128, QB], f32, tag="oa")
                        ps_ob = psO.tile([128, QB], f32, tag="ob")

                    def pv_all():
                        for c in range(NCH):
                            kc, hb = c // 2, c % 2
                            ps_o = ps_oa if hb == 0 else ps_ob
                            nc.tensor.matmul(
                                ps_o[0 : DK + 1, :],
                                v_t[:, kc, 2 * php + hb, :],
                                pexp[:, c, :],
                                start=(kc == 0),
                                stop=(kc == NKC - 1),
                            )

                    def score_group(g0, g1):
                        ps = psS.tile([128, 3, QB], f32, tag="s", name="ps_s")
                        for i, c in enumerate(range(g0, g1)):
                            kc, hb = c // 2, c % 2
                            p0 = hb * 64
                            k_sl = slice(kc * 128, (kc + 1) * 128)
                            nc.tensor.matmul(
                                ps[:, i],
                                kt_t[p0 : p0 + 64, hp, k_sl],
                                qt_t[p0 : p0 + 64, hp, q_sl],
                                start=True,
                                stop=True,
                                tile_position=(p0, 0),
                            )
                        nc.scalar.activation(
                            out=expst[:, g0:g1, :],
                            in_=ps[:, 0 : g1 - g0, :],
                            func=mybir.ActivationFunctionType.Exp,
                            scale=SCALE_INV,
                        )

                    if prev is not None:
                        pv_all()
                        # normalize immediately: its short engine-only chain
                        # runs under the score stream, so the out-proj
                        # matmuls emitted later never wait on it
                        emit_normalize(pq, php, ps_oa, ps_ob)

                    if not final:
                        for g0, g1 in GRPS:
                            score_group(g0, g1)
                        if prev is not None and php == 1:
                            # defer part of late query blocks' output
                            # projection into the final step, where it
                            # covers the last normalize chain's latency
                            hi = {0: 8, 1: 6, 2: 4}[pq]
                            emit_outproj(pq, 0, hi)
                        return expst

                    # final step: chase the exp stream with our own PV
                    ps_fa = psO.tile([128, QB], f32, tag="oa")
                    ps_fb = psO.tile([128, QB], f32, tag="ob")

                    def pv_self(g0, g1):
                        for c in range(g0, g1):
                            kc, hb = c // 2, c % 2
                            ps_o = ps_fa if hb == 0 else ps_fb
                            nc.tensor.matmul(
                                ps_o[0 : DK + 1, :],
                                v_t[:, kc, 2 * hp + hb, :],
                                expst[:, c, :],
                                start=(kc == 0),
                                stop=(kc == NKC - 1),
                            )

                    for gi, (g0, g1) in enumerate(GRPS):
                        score_group(g0, g1)
                        if gi >= 1:
                            pv_self(*GRPS[gi - 1])
                    pv_self(*GRPS[-1])
                    emit_normalize(qb, hp, ps_fa, ps_fb)
                    # deferred late-block slots execute while the final z
                    # chain is in flight (y copies on the now-idle ACT)
                    emit_outproj(1, 6, 8, copies_on_act=True)
                    emit_outproj(2, 4, 8, copies_on_act=True)
                    emit_outproj(qb, copies_on_act=True)
                    return expst

                steps = [(qb, hp) for qb in range(NQB) for hp in range(2)]
                prev = None
                for i, (qb, hp) in enumerate(steps):
                    expst = emit_step(
                        qb, hp, prev, final=(i == len(steps) - 1)
                    )
                    prev = (qb, hp, expst)

    nc.compile()
    return nc


def kernel(X, W_Q, W_K, W_V, W_0):
    global LAST_RESULT
    from concourse.bass_utils import run_bass_kernel_spmd
    import os

    bf = ml_dtypes.bfloat16
    X = np.asarray(X, dtype=np.float32)
    W_Q = np.asarray(W_Q, dtype=np.float32).astype(bf)
    W_K = np.asarray(W_K, dtype=np.float32).astype(bf)
    W_V = np.asarray(W_V, dtype=np.float32).astype(bf)
    W_0 = np.asarray(W_0, dtype=np.float32).astype(bf)

    if "nc" not in _CACHE:
        _CACHE["nc"] = _build()
    nc = _CACHE["nc"]

    xt = [np.ascontiguousarray(X[b].T).astype(bf) for b in range(B)]
    in_maps = []
    for c in range(NCORES):
        b, g = c // HG, c % HG
        js = slice(g * J, (g + 1) * J)
        in_maps.append(
            {
                "xt": xt[b],
                "wq": np.ascontiguousarray(W_Q[:, js]),
                "wk": np.ascontiguousarray(W_K[:, js]),
                "wv": np.ascontiguousarray(W_V[:, js]),
                "w0": np.ascontiguousarray(W_0[js, :]),
            }
        )

    trace = bool(int(os.environ.get("KERNEL_TRACE", "0")))
    res = run_bass_kernel_spmd(
        nc, in_maps, list(range(NCORES)), trace=trace
    )
    LAST_RESULT = res

    out = np.zeros((B, S, D), dtype=np.float32)
    for c in range(NCORES):
        out[c // HG] += res.results[c]["y"].astype(np.float32)
    return out
